# revision 2
# baseline (speedup 1.0000x reference)
"""Trainium2 Bass kernel for nn_ClusteringLayer (vq_codebook).

q[n,k] = t / sum_k t,  t = 1/(1 + ||x_n - c_k||^2)   (Student-t, alpha=1)

Strategy (8 NeuronCores, data-parallel over N):
  - shard x along N (32768 rows/core); replicate the (256,64) cluster table.
  - per core, process 1024-row chunks: 8 subtiles of 128 rows.
  - squared distances via PE matmuls accumulating into PSUM:
      S = (1+|c|^2)[k] + |x|^2[n] - 2 x.c
      mm1: lhsT=xT       rhs=-2*c^T      (K=64)
      mm2: lhsT=(x^2)T   rhs=ones[64,256](K=64)   -> adds |x|^2[n]
      mm3: lhsT=ones[1,128] rhs=(1+|c|^2)[1,256] (K=1)
    xT tiles come from PE pair-transposes ([128,128] -> PSUM), copied to
    SBUF by ScalarE (plain copy -> xT, Square-activation copy -> (x^2)T).
  - q_unnorm = 1/S via single-instruction DVE approx reciprocal (PSUM src).
  - rowsums on DVE (3D tensor_reduce), exact DVE reciprocal of the sums,
    per-row scale via tensor_scalar (2x mode), DMA out.
"""

import sys

sys.path.insert(0, "/opt/trn_rl_repo")

import numpy as np

N, D, K = 262144, 64, 256
NCORES = 8
NS = N // NCORES  # rows per core
CHUNK = 1024      # rows per chunk
G = 8             # subtiles (of 128 rows) per chunk
NCHUNK = NS // CHUNK

_CACHE = {}


def _build_program():
    import concourse.bacc as bacc
    import concourse.tile as tile
    from concourse import mybir

    f32 = mybir.dt.float32
    nc = bacc.Bacc("TRN2", target_bir_lowering=False, debug=False)

    x_ap = nc.dram_tensor("x", [NS, D], f32, kind="ExternalInput").ap()
    w1_ap = nc.dram_tensor("w1", [2 * D, K], f32, kind="ExternalInput").ap()
    ones64_ap = nc.dram_tensor("ones64", [2 * D, K], f32, kind="ExternalInput").ap()
    csq_ap = nc.dram_tensor("csq", [1, K], f32, kind="ExternalInput").ap()
    ones1_ap = nc.dram_tensor("ones1", [1, 128], f32, kind="ExternalInput").ap()
    ident_ap = nc.dram_tensor("ident", [128, 128], f32, kind="ExternalInput").ap()
    q_ap = nc.dram_tensor("q", [NS, K], f32, kind="ExternalOutput").ap()

    with tile.TileContext(nc) as tc:
        _body(nc, tc, mybir, x_ap, w1_ap, ones64_ap, csq_ap, ones1_ap, ident_ap, q_ap)
    nc.compile()
    return nc


def _body(nc, tc, mybir, x_ap, w1_ap, ones64_ap, csq_ap, ones1_ap, ident_ap, q_ap):
    from contextlib import ExitStack

    f32 = mybir.dt.float32
    ctx = ExitStack()
    with ctx:
        consts = ctx.enter_context(tc.tile_pool(name="consts", bufs=1))
        w1 = consts.tile([2 * D, K], f32)
        nc.sync.dma_start(w1[:], w1_ap[:])
        ones64 = consts.tile([2 * D, K], f32)
        nc.sync.dma_start(ones64[:], ones64_ap[:])
        csq = consts.tile([1, K], f32)
        nc.sync.dma_start(csq[:], csq_ap[:])
        ones1 = consts.tile([1, 128], f32)
        nc.sync.dma_start(ones1[:], ones1_ap[:])
        ident = consts.tile([128, 128], f32)
        nc.sync.dma_start(ident[:], ident_ap[:])

        xp = ctx.enter_context(tc.tile_pool(name="xp", bufs=3))
        tpp = ctx.enter_context(tc.tile_pool(name="tpp", bufs=3, space="PSUM"))
        lp = ctx.enter_context(tc.tile_pool(name="lp", bufs=3))
        sp = ctx.enter_context(tc.tile_pool(name="sp", bufs=3))
        qp = ctx.enter_context(tc.tile_pool(name="qp", bufs=2, space="PSUM"))
        qup = ctx.enter_context(tc.tile_pool(name="qup", bufs=3))
        rsp = ctx.enter_context(tc.tile_pool(name="rsp", bufs=2))
        qop = ctx.enter_context(tc.tile_pool(name="qop", bufs=2))

        for c in range(NCHUNK):
            r0 = c * CHUNK
            xc = xp.tile([128, CHUNK // 128 * D], f32)  # [128, 512]
            nc.sync.dma_start(
                xc[:],
                x_ap[r0 : r0 + CHUNK, :].rearrange("(p g) d -> p (g d)", p=128),
            )
            rs = rsp.tile([128, G], f32)
            rr = rsp.tile([128, G], f32)
            qout = qop.tile([128, G * K], f32)  # [128, 2048]
            qus = []
            for hc in range(2):  # half-chunks of 4 subtiles
                psq = qp.tile([128, 4 * K], f32)  # [128, 1024] = 2 banks
                for t2 in range(2):  # transpose pairs in this half-chunk
                    t = 2 * hc + t2
                    pst = tpp.tile([128, 128], f32)
                    nc.tensor.transpose(pst[:], xc[:, 128 * t : 128 * (t + 1)], ident[:])
                    lt = lp.tile([128, 128], f32)
                    nc.scalar.copy(lt[:], pst[:])
                    st = sp.tile([128, 128], f32)
                    nc.scalar.activation(
                        st[:], pst[:], mybir.ActivationFunctionType.Square
                    )
                    for h in range(2):  # subtiles in pair
                        col = 512 * t2 + 256 * h
                        out_ap = psq[:, col : col + K]
                        nc.tensor.matmul(
                            out_ap, lt[64 * h : 64 * h + 64, :], w1[64 * h : 64 * h + 64, :],
                            start=True, stop=False,
                        )
                        nc.tensor.matmul(
                            out_ap, st[64 * h : 64 * h + 64, :], ones64[64 * h : 64 * h + 64, :],
                            start=False, stop=False,
                        )
                        nc.tensor.matmul(
                            out_ap, ones1[:], csq[:],
                            start=False, stop=True,
                        )
                qu = qup.tile([128, 4 * K], f32)
                nc.vector.reciprocal_approx_fast(out=qu[:], in_=psq[:])
                nc.vector.tensor_reduce(
                    rs[:, 4 * hc : 4 * hc + 4],
                    qu[:].rearrange("p (s k) -> p s k", k=K),
                    axis=mybir.AxisListType.X,
                    op=mybir.AluOpType.add,
                )
                qus.append(qu)
            nc.vector.reciprocal(rr[:], rs[:])
            for j in range(G):
                nc.vector.tensor_scalar_mul(
                    qout[:, K * j : K * (j + 1)],
                    qus[j // 4][:, K * (j % 4) : K * (j % 4 + 1)],
                    rr[:, j : j + 1],
                )
            nc.sync.dma_start(
                q_ap[r0 : r0 + CHUNK, :].rearrange("(p g) k -> p (g k)", p=128),
                qout[:],
            )


def _get_program():
    if "nc" not in _CACHE:
        _CACHE["nc"] = _build_program()
    return _CACHE["nc"]


def kernel(x, clusters):
    from concourse.bass_utils import run_bass_kernel_spmd

    x = np.ascontiguousarray(np.asarray(x, dtype=np.float32))
    clusters = np.ascontiguousarray(np.asarray(clusters, dtype=np.float32))
    assert x.shape == (N, D) and clusters.shape == (K, D)

    nc = _get_program()

    w1half = (-2.0 * clusters.T).astype(np.float32)
    w1 = np.ascontiguousarray(np.vstack([w1half, w1half]))
    csq = (1.0 + np.sum(clusters * clusters, axis=1, dtype=np.float32)).reshape(1, K)
    consts = {
        "w1": w1,
        "ones64": np.ones((2 * D, K), dtype=np.float32),
        "csq": np.ascontiguousarray(csq.astype(np.float32)),
        "ones1": np.ones((1, 128), dtype=np.float32),
        "ident": np.eye(128, dtype=np.float32),
    }
    in_maps = [
        {"x": np.ascontiguousarray(x[i * NS : (i + 1) * NS]), **consts}
        for i in range(NCORES)
    ]
    res = run_bass_kernel_spmd(nc, in_maps, core_ids=list(range(NCORES)))
    out = np.concatenate([res.results[i]["q"] for i in range(NCORES)], axis=0)
    return out


# revision 3
# speedup vs baseline: 10127.1113x; 10127.1113x over previous
"""Trainium2 Bass kernel for nn_ClusteringLayer (vq_codebook).

q[n,k] = t / sum_k t,  t = 1/(1 + ||x_n - c_k||^2)   (Student-t, alpha=1)

Strategy (8 NeuronCores, data-parallel over N):
  - shard x along N (32768 rows/core); replicate the (256,64) cluster table.
  - per core, process 1024-row chunks: 8 subtiles of 128 rows.
  - squared distances via PE matmuls accumulating into PSUM:
      S = (1+|c|^2)[k] + |x|^2[n] - 2 x.c
      mm1: lhsT=xT       rhs=-2*c^T      (K=64)
      mm2: lhsT=(x^2)T   rhs=ones[64,256](K=64)   -> adds |x|^2[n]
      mm3: lhsT=ones[1,128] rhs=(1+|c|^2)[1,256] (K=1)
    xT tiles come from PE pair-transposes ([128,128] -> PSUM), copied to
    SBUF by ScalarE (plain copy -> xT, Square-activation copy -> (x^2)T).
  - q_unnorm = 1/S via single-instruction DVE approx reciprocal (PSUM src).
  - rowsums on DVE (3D tensor_reduce), exact DVE reciprocal of the sums,
    per-row scale via tensor_scalar (2x mode), DMA out.
"""

import sys

sys.path.insert(0, "/opt/trn_rl_repo")

import numpy as np

N, D, K = 262144, 64, 256
NCORES = 8
NS = N // NCORES  # rows per core
CHUNK = 1024      # rows per chunk
G = 8             # subtiles (of 128 rows) per chunk
NCHUNK = NS // CHUNK

_CACHE = {}


def _build_program(loop_reps=None):
    import concourse.bacc as bacc
    import concourse.tile as tile
    from concourse import mybir

    f32 = mybir.dt.float32
    nc = bacc.Bacc("TRN2", target_bir_lowering=False, debug=False)

    x_ap = nc.dram_tensor("x", [NS, D], f32, kind="ExternalInput").ap()
    w1_ap = nc.dram_tensor("w1", [2 * D, K], f32, kind="ExternalInput").ap()
    ones64_ap = nc.dram_tensor("ones64", [2 * D, K], f32, kind="ExternalInput").ap()
    csq_ap = nc.dram_tensor("csq", [1, K], f32, kind="ExternalInput").ap()
    ones1_ap = nc.dram_tensor("ones1", [1, 128], f32, kind="ExternalInput").ap()
    ident_ap = nc.dram_tensor("ident", [128, 128], f32, kind="ExternalInput").ap()
    q_ap = nc.dram_tensor("q", [NS, K], f32, kind="ExternalOutput").ap()

    with tile.TileContext(nc) as tc:
        if loop_reps is None:
            _body(nc, tc, mybir, x_ap, w1_ap, ones64_ap, csq_ap, ones1_ap, ident_ap, q_ap)
        else:
            with tc.For_i(0, loop_reps, 1):
                _body(nc, tc, mybir, x_ap, w1_ap, ones64_ap, csq_ap, ones1_ap, ident_ap, q_ap)
    nc.compile()
    return nc


def _body(nc, tc, mybir, x_ap, w1_ap, ones64_ap, csq_ap, ones1_ap, ident_ap, q_ap):
    from contextlib import ExitStack

    f32 = mybir.dt.float32
    ctx = ExitStack()
    with ctx:
        consts = ctx.enter_context(tc.tile_pool(name="consts", bufs=1))
        w1 = consts.tile([2 * D, K], f32)
        nc.sync.dma_start(w1[:], w1_ap[:])
        ones64 = consts.tile([2 * D, K], f32)
        nc.sync.dma_start(ones64[:], ones64_ap[:])
        csq = consts.tile([1, K], f32)
        nc.sync.dma_start(csq[:], csq_ap[:])
        ones1 = consts.tile([1, 128], f32)
        nc.sync.dma_start(ones1[:], ones1_ap[:])
        ident = consts.tile([128, 128], f32)
        nc.sync.dma_start(ident[:], ident_ap[:])

        xp = ctx.enter_context(tc.tile_pool(name="xp", bufs=3))
        tpp = ctx.enter_context(tc.tile_pool(name="tpp", bufs=3, space="PSUM"))
        lp = ctx.enter_context(tc.tile_pool(name="lp", bufs=3))
        sp = ctx.enter_context(tc.tile_pool(name="sp", bufs=3))
        qp = ctx.enter_context(tc.tile_pool(name="qp", bufs=2, space="PSUM"))
        qup = ctx.enter_context(tc.tile_pool(name="qup", bufs=3))
        rsp = ctx.enter_context(tc.tile_pool(name="rsp", bufs=2))
        qop = ctx.enter_context(tc.tile_pool(name="qop", bufs=2))

        for c in range(NCHUNK):
            r0 = c * CHUNK
            xc = xp.tile([128, CHUNK // 128 * D], f32)  # [128, 512]
            nc.sync.dma_start(
                xc[:],
                x_ap[r0 : r0 + CHUNK, :].rearrange("(p g) d -> p (g d)", p=128),
            )
            rs = rsp.tile([128, G], f32)
            rr = rsp.tile([128, G], f32)
            qout = qop.tile([128, G * K], f32)  # [128, 2048]
            qus = []
            for hc in range(2):  # half-chunks of 4 subtiles
                psq = qp.tile([128, 4 * K], f32)  # [128, 1024] = 2 banks
                for t2 in range(2):  # transpose pairs in this half-chunk
                    t = 2 * hc + t2
                    pst = tpp.tile([128, 128], f32)
                    nc.tensor.transpose(pst[:], xc[:, 128 * t : 128 * (t + 1)], ident[:])
                    lt = lp.tile([128, 128], f32)
                    nc.scalar.copy(lt[:], pst[:])
                    st = sp.tile([128, 128], f32)
                    nc.scalar.activation(
                        st[:], pst[:], mybir.ActivationFunctionType.Square
                    )
                    for h in range(2):  # subtiles in pair
                        col = 512 * t2 + 256 * h
                        out_ap = psq[:, col : col + K]
                        nc.tensor.matmul(
                            out_ap, lt[64 * h : 64 * h + 64, :], w1[64 * h : 64 * h + 64, :],
                            start=True, stop=False,
                        )
                        nc.tensor.matmul(
                            out_ap, st[64 * h : 64 * h + 64, :], ones64[64 * h : 64 * h + 64, :],
                            start=False, stop=False,
                        )
                        nc.tensor.matmul(
                            out_ap, ones1[:], csq[:],
                            start=False, stop=True,
                        )
                qu = qup.tile([128, 4 * K], f32)
                nc.vector.reciprocal_approx_fast(out=qu[:], in_=psq[:])
                nc.vector.tensor_reduce(
                    rs[:, 4 * hc : 4 * hc + 4],
                    qu[:].rearrange("p (s k) -> p s k", k=K),
                    axis=mybir.AxisListType.X,
                    op=mybir.AluOpType.add,
                )
                qus.append(qu)
            nc.vector.reciprocal(rr[:], rs[:])
            for j in range(G):
                nc.vector.tensor_scalar_mul(
                    qout[:, K * j : K * (j + 1)],
                    qus[j // 4][:, K * (j % 4) : K * (j % 4 + 1)],
                    rr[:, j : j + 1],
                )
            nc.sync.dma_start(
                q_ap[r0 : r0 + CHUNK, :].rearrange("(p g) k -> p (g k)", p=128),
                qout[:],
            )


def _get_program():
    if "nc" not in _CACHE:
        _CACHE["nc"] = _build_program()
    return _CACHE["nc"]


def kernel(x, clusters):
    from concourse.bass_utils import run_bass_kernel_spmd

    x = np.ascontiguousarray(np.asarray(x, dtype=np.float32))
    clusters = np.ascontiguousarray(np.asarray(clusters, dtype=np.float32))
    assert x.shape == (N, D) and clusters.shape == (K, D)

    nc = _get_program()

    w1half = (-2.0 * clusters.T).astype(np.float32)
    w1 = np.ascontiguousarray(np.vstack([w1half, w1half]))
    csq = (1.0 + np.sum(clusters * clusters, axis=1, dtype=np.float32)).reshape(1, K)
    consts = {
        "w1": w1,
        "ones64": np.ones((2 * D, K), dtype=np.float32),
        "csq": np.ascontiguousarray(csq.astype(np.float32)),
        "ones1": np.ones((1, 128), dtype=np.float32),
        "ident": np.eye(128, dtype=np.float32),
    }
    in_maps = [
        {"x": np.ascontiguousarray(x[i * NS : (i + 1) * NS]), **consts}
        for i in range(NCORES)
    ]
    res = run_bass_kernel_spmd(nc, in_maps, core_ids=list(range(NCORES)))
    out = np.concatenate([res.results[i]["q"] for i in range(NCORES)], axis=0)
    return out


# revision 6
# speedup vs baseline: 15844.1853x; 1.5645x over previous
"""Trainium2 Bass kernel for nn_ClusteringLayer (vq_codebook).

q[n,k] = t / sum_k t,  t = 1/(1 + ||x_n - c_k||^2)   (Student-t, alpha=1)

Strategy (8 NeuronCores, data-parallel over N):
  - shard x along N (32768 rows/core); replicate the (256,64) cluster table.
  - per core, process 1024-row chunks: 8 subtiles of 128 rows.
  - squared distances via PE matmuls accumulating into PSUM:
      S = (1+|c|^2)[k] + |x|^2[n] - 2 x.c
      mm1: lhsT=xT       rhs=-2*c^T      (K=64)
      mm2: lhsT=(x^2)T   rhs=ones[64,256](K=64)   -> adds |x|^2[n]
      mm3: lhsT=ones[1,128] rhs=(1+|c|^2)[1,256] (K=1)
    xT tiles come from PE pair-transposes ([128,128] -> PSUM), copied to
    SBUF by ScalarE (plain copy -> xT, Square-activation copy -> (x^2)T).
  - q_unnorm = 1/S via single-instruction DVE approx reciprocal (PSUM src).
  - rowsums on DVE (3D tensor_reduce), exact DVE reciprocal of the sums,
    per-row scale via tensor_scalar (2x mode), DMA out.
"""

import sys

sys.path.insert(0, "/opt/trn_rl_repo")

import numpy as np

N, D, K = 262144, 64, 256
NCORES = 8
NS = N // NCORES  # rows per core
CHUNK = 1024      # rows per chunk
G = 8             # subtiles (of 128 rows) per chunk
NCHUNK = NS // CHUNK

_CACHE = {}


def _build_program(loop_reps=None):
    import concourse.bacc as bacc
    import concourse.tile as tile
    from concourse import mybir

    f32 = mybir.dt.float32
    nc = bacc.Bacc("TRN2", target_bir_lowering=False, debug=False)

    x_ap = nc.dram_tensor("x", [NS, D], f32, kind="ExternalInput").ap()
    w1_ap = nc.dram_tensor("w1", [2 * D, K], f32, kind="ExternalInput").ap()
    ones64_ap = nc.dram_tensor("ones64", [2 * D, K], f32, kind="ExternalInput").ap()
    csq_ap = nc.dram_tensor("csq", [1, 2 * K], f32, kind="ExternalInput").ap()
    ones1_ap = nc.dram_tensor("ones1", [1, 128], f32, kind="ExternalInput").ap()
    ident_ap = nc.dram_tensor("ident", [128, 128], f32, kind="ExternalInput").ap()
    q_ap = nc.dram_tensor("q", [NS, K], f32, kind="ExternalOutput").ap()

    with tile.TileContext(nc) as tc:
        if loop_reps is None:
            _body(nc, tc, mybir, x_ap, w1_ap, ones64_ap, csq_ap, ones1_ap, ident_ap, q_ap)
        else:
            with tc.For_i(0, loop_reps, 1):
                _body(nc, tc, mybir, x_ap, w1_ap, ones64_ap, csq_ap, ones1_ap, ident_ap, q_ap)
    nc.compile()
    return nc


def _body(nc, tc, mybir, x_ap, w1_ap, ones64_ap, csq_ap, ones1_ap, ident_ap, q_ap):
    from contextlib import ExitStack

    f32 = mybir.dt.float32
    ctx = ExitStack()
    with ctx:
        consts = ctx.enter_context(tc.tile_pool(name="consts", bufs=1))
        w1 = consts.tile([2 * D, K], f32)
        nc.sync.dma_start(w1[:], w1_ap[:])
        ones64 = consts.tile([2 * D, K], f32)
        nc.sync.dma_start(ones64[:], ones64_ap[:])
        csq2 = consts.tile([1, 2 * K], f32)
        nc.sync.dma_start(csq2[:], csq_ap[:])
        ones1 = consts.tile([1, 128], f32)
        nc.sync.dma_start(ones1[:], ones1_ap[:])
        ident = consts.tile([128, 128], f32)
        nc.sync.dma_start(ident[:], ident_ap[:])

        xp = ctx.enter_context(tc.tile_pool(name="xp", bufs=3))
        tpp = ctx.enter_context(tc.tile_pool(name="tpp", bufs=3, space="PSUM"))
        lp = ctx.enter_context(tc.tile_pool(name="lp", bufs=3))
        sp = ctx.enter_context(tc.tile_pool(name="sp", bufs=3))
        qp = ctx.enter_context(tc.tile_pool(name="qp", bufs=2, space="PSUM"))
        qup = ctx.enter_context(tc.tile_pool(name="qup", bufs=3))
        rsp = ctx.enter_context(tc.tile_pool(name="rsp", bufs=2))
        qop = ctx.enter_context(tc.tile_pool(name="qop", bufs=2))

        for c in range(NCHUNK):
            r0 = c * CHUNK
            xc = xp.tile([128, CHUNK // 128 * D], f32)  # [128, 512]
            nc.sync.dma_start(
                xc[:],
                x_ap[r0 : r0 + CHUNK, :].rearrange("(p g) d -> p (g d)", p=128),
            )
            rs = rsp.tile([128, G], f32)
            rr = rsp.tile([128, G], f32)
            qout = qop.tile([128, G * K], f32)  # [128, 2048]
            qus = []
            # within a half-chunk's [128,1024] psum tile (2 banks), place the
            # two subtiles of each transpose-pair in DIFFERENT banks so their
            # row-tiled matmuls (tile_position rows 0/64) can run concurrently.
            COL = {0: 0, 1: 512, 2: 256, 3: 768}
            for hc in range(2):  # half-chunks of 4 subtiles
                psq = qp.tile([128, 4 * K], f32)  # [128, 1024] = 2 banks
                # start=True clears has_written for the WHOLE bank, so each
                # bank's group must open with exactly one start=True matmul
                # covering it: the (1+|c|^2) rank-1 over all 512 columns.
                for b in range(2):
                    nc.tensor.matmul(
                        psq[:, 512 * b : 512 * (b + 1)], ones1[:], csq2[:],
                        start=True, stop=False, skip_group_check=True,
                    )
                for t2 in range(2):  # transpose pairs in this half-chunk
                    t = 2 * hc + t2
                    pst = tpp.tile([128, 128], f32)
                    nc.tensor.transpose(pst[:], xc[:, 128 * t : 128 * (t + 1)], ident[:])
                    lt = lp.tile([128, 128], f32)
                    nc.scalar.copy(lt[:], pst[:])
                    st = sp.tile([128, 128], f32)
                    nc.scalar.activation(
                        st[:], pst[:], mybir.ActivationFunctionType.Square
                    )
                    cols = [COL[2 * t2], COL[2 * t2 + 1]]
                    for h in range(2):  # cross terms, adjacent for row-packing
                        nc.tensor.matmul(
                            psq[:, cols[h] : cols[h] + K],
                            lt[64 * h : 64 * h + 64, :], w1[64 * h : 64 * h + 64, :],
                            start=False, stop=False, tile_position=(64 * h, 0),
                            skip_group_check=True,
                        )
                    for h in range(2):  # |x|^2 terms; last pair closes banks
                        nc.tensor.matmul(
                            psq[:, cols[h] : cols[h] + K],
                            st[64 * h : 64 * h + 64, :], ones64[64 * h : 64 * h + 64, :],
                            start=False, stop=(t2 == 1), tile_position=(64 * h, 0),
                            skip_group_check=True,
                        )
                qu = qup.tile([128, 4 * K], f32)
                nc.vector.reciprocal_approx_fast(out=qu[:], in_=psq[:])
                nc.vector.tensor_reduce(
                    rs[:, 4 * hc : 4 * hc + 4],
                    qu[:].rearrange("p (s k) -> p s k", k=K),
                    axis=mybir.AxisListType.X,
                    op=mybir.AluOpType.add,
                )
                qus.append(qu)
            nc.vector.reciprocal(rr[:], rs[:])
            BLK = {0: 0, 1: 2, 2: 1, 3: 3}  # jl -> reduce block index
            for j in range(G):
                hc, jl = j // 4, j % 4
                nc.vector.tensor_scalar_mul(
                    qout[:, K * j : K * (j + 1)],
                    qus[hc][:, COL[jl] : COL[jl] + K],
                    rr[:, 4 * hc + BLK[jl] : 4 * hc + BLK[jl] + 1],
                )
            nc.sync.dma_start(
                q_ap[r0 : r0 + CHUNK, :].rearrange("(p g) k -> p (g k)", p=128),
                qout[:],
            )


def _get_program():
    if "nc" not in _CACHE:
        _CACHE["nc"] = _build_program()
    return _CACHE["nc"]


def kernel(x, clusters):
    from concourse.bass_utils import run_bass_kernel_spmd

    x = np.ascontiguousarray(np.asarray(x, dtype=np.float32))
    clusters = np.ascontiguousarray(np.asarray(clusters, dtype=np.float32))
    assert x.shape == (N, D) and clusters.shape == (K, D)

    nc = _get_program()

    w1half = (-2.0 * clusters.T).astype(np.float32)
    w1 = np.ascontiguousarray(np.vstack([w1half, w1half]))
    csq1 = (1.0 + np.sum(clusters * clusters, axis=1, dtype=np.float32)).reshape(1, K)
    csq = np.tile(csq1, (1, 2))
    consts = {
        "w1": w1,
        "ones64": np.ones((2 * D, K), dtype=np.float32),
        "csq": np.ascontiguousarray(csq.astype(np.float32)),
        "ones1": np.ones((1, 128), dtype=np.float32),
        "ident": np.eye(128, dtype=np.float32),
    }
    in_maps = [
        {"x": np.ascontiguousarray(x[i * NS : (i + 1) * NS]), **consts}
        for i in range(NCORES)
    ]
    res = run_bass_kernel_spmd(nc, in_maps, core_ids=list(range(NCORES)))
    out = np.concatenate([res.results[i]["q"] for i in range(NCORES)], axis=0)
    return out


# revision 7
# speedup vs baseline: 16981.3752x; 1.0718x over previous
"""Trainium2 Bass kernel for nn_ClusteringLayer (vq_codebook).

q[n,k] = t / sum_k t,  t = 1/(1 + ||x_n - c_k||^2)   (Student-t, alpha=1)

Strategy (8 NeuronCores, data-parallel over N):
  - shard x along N (32768 rows/core); replicate the (256,64) cluster table.
  - per core, process 1024-row chunks: 8 subtiles of 128 rows.
  - squared distances via PE matmuls accumulating into PSUM:
      S = (1+|c|^2)[k] + |x|^2[n] - 2 x.c
      mm1: lhsT=xT       rhs=-2*c^T      (K=64)
      mm2: lhsT=(x^2)T   rhs=ones[64,256](K=64)   -> adds |x|^2[n]
      mm3: lhsT=ones[1,128] rhs=(1+|c|^2)[1,256] (K=1)
    xT tiles come from PE pair-transposes ([128,128] -> PSUM), copied to
    SBUF by ScalarE (plain copy -> xT, Square-activation copy -> (x^2)T).
  - q_unnorm = 1/S via single-instruction DVE approx reciprocal (PSUM src).
  - rowsums on DVE (3D tensor_reduce), exact DVE reciprocal of the sums,
    per-row scale via tensor_scalar (2x mode), DMA out.
"""

import sys

sys.path.insert(0, "/opt/trn_rl_repo")

import numpy as np

N, D, K = 262144, 64, 256
NCORES = 8
NS = N // NCORES  # rows per core
CHUNK = 1024      # rows per chunk
G = 8             # subtiles (of 128 rows) per chunk
NCHUNK = NS // CHUNK

_CACHE = {}


def _build_program(loop_reps=None):
    import concourse.bacc as bacc
    import concourse.tile as tile
    from concourse import mybir

    f32 = mybir.dt.float32
    nc = bacc.Bacc("TRN2", target_bir_lowering=False, debug=False)

    x_ap = nc.dram_tensor("x", [NS, D], f32, kind="ExternalInput").ap()
    w1_ap = nc.dram_tensor("w1", [128, K], f32, kind="ExternalInput").ap()
    csq_ap = nc.dram_tensor("csq", [1, 2 * K], f32, kind="ExternalInput").ap()
    ones1_ap = nc.dram_tensor("ones1", [1, 128], f32, kind="ExternalInput").ap()
    ident_ap = nc.dram_tensor("ident", [128, 128], f32, kind="ExternalInput").ap()
    q_ap = nc.dram_tensor("q", [NS, K], f32, kind="ExternalOutput").ap()

    with tile.TileContext(nc) as tc:
        if loop_reps is None:
            _body(nc, tc, mybir, x_ap, w1_ap, csq_ap, ones1_ap, ident_ap, q_ap)
        else:
            with tc.For_i(0, loop_reps, 1):
                _body(nc, tc, mybir, x_ap, w1_ap, csq_ap, ones1_ap, ident_ap, q_ap)
    nc.compile()
    return nc


def _body(nc, tc, mybir, x_ap, w1_ap, csq_ap, ones1_ap, ident_ap, q_ap):
    from contextlib import ExitStack

    f32 = mybir.dt.float32
    ctx = ExitStack()
    with ctx:
        consts = ctx.enter_context(tc.tile_pool(name="consts", bufs=1))
        w1 = consts.tile([128, K], f32)   # [-2 c^T ; ones(64,256)]
        nc.sync.dma_start(w1[:], w1_ap[:])
        csq2 = consts.tile([1, 2 * K], f32)
        nc.sync.dma_start(csq2[:], csq_ap[:])
        ones1 = consts.tile([1, 128], f32)
        nc.sync.dma_start(ones1[:], ones1_ap[:])
        ident = consts.tile([128, 128], f32)
        nc.sync.dma_start(ident[:], ident_ap[:])

        xp = ctx.enter_context(tc.tile_pool(name="xp", bufs=3))
        tpp = ctx.enter_context(tc.tile_pool(name="tpp", bufs=3, space="PSUM"))
        lp = ctx.enter_context(tc.tile_pool(name="lp", bufs=3))
        qp = ctx.enter_context(tc.tile_pool(name="qp", bufs=2, space="PSUM"))
        qup = ctx.enter_context(tc.tile_pool(name="qup", bufs=3))
        rsp = ctx.enter_context(tc.tile_pool(name="rsp", bufs=2))
        qop = ctx.enter_context(tc.tile_pool(name="qop", bufs=2))

        for c in range(NCHUNK):
            r0 = c * CHUNK
            # XB holds per subtile j a [x_j | x_j^2] 128-column block.
            xb = xp.tile([128, G * 128], f32)  # [128, 1024]
            xb3 = xb[:].rearrange("p (g td) -> p g td", td=128)
            nc.sync.dma_start(
                xb3[:, :, 0:64],
                x_ap[r0 : r0 + CHUNK, :].rearrange("(p g) d -> p g d", p=128),
            )
            nc.scalar.activation(
                xb3[:, :, 64:128], xb3[:, :, 0:64],
                mybir.ActivationFunctionType.Square,
            )
            rs = rsp.tile([128, G], f32)
            rr = rsp.tile([128, G], f32)
            qout = qop.tile([128, G * K], f32)  # [128, 2048]
            qus = []
            for hc in range(2):  # half-chunks of 4 subtiles
                psq = qp.tile([128, 4 * K], f32)  # [128, 1024] = 2 banks
                # start=True clears has_written for the WHOLE bank: open each
                # bank group with one full-bank (1+|c|^2) rank-1 matmul.
                for b in range(2):
                    nc.tensor.matmul(
                        psq[:, 512 * b : 512 * (b + 1)], ones1[:], csq2[:],
                        start=True, stop=False, skip_group_check=True,
                    )
                for jl in range(4):
                    j = 4 * hc + jl
                    pst = tpp.tile([128, 128], f32)
                    nc.tensor.transpose(pst[:], xb[:, 128 * j : 128 * (j + 1)], ident[:])
                    lt = lp.tile([128, 128], f32)
                    nc.scalar.copy(lt[:], pst[:])
                    nc.tensor.matmul(
                        psq[:, K * jl : K * (jl + 1)], lt[:], w1[:],
                        start=False, stop=(jl >= 2), skip_group_check=True,
                    )
                qu = qup.tile([128, 4 * K], f32)
                nc.vector.reciprocal_approx_fast(out=qu[:], in_=psq[:])
                nc.vector.tensor_reduce(
                    rs[:, 4 * hc : 4 * hc + 4],
                    qu[:].rearrange("p (s k) -> p s k", k=K),
                    axis=mybir.AxisListType.X,
                    op=mybir.AluOpType.add,
                )
                qus.append(qu)
            nc.vector.reciprocal(rr[:], rs[:])
            for j in range(G):
                hc, jl = j // 4, j % 4
                nc.vector.tensor_scalar_mul(
                    qout[:, K * j : K * (j + 1)],
                    qus[hc][:, K * jl : K * (jl + 1)],
                    rr[:, j : j + 1],
                )
            nc.sync.dma_start(
                q_ap[r0 : r0 + CHUNK, :].rearrange("(p g) k -> p (g k)", p=128),
                qout[:],
            )


def _get_program():
    if "nc" not in _CACHE:
        _CACHE["nc"] = _build_program()
    return _CACHE["nc"]


def kernel(x, clusters):
    from concourse.bass_utils import run_bass_kernel_spmd

    x = np.ascontiguousarray(np.asarray(x, dtype=np.float32))
    clusters = np.ascontiguousarray(np.asarray(clusters, dtype=np.float32))
    assert x.shape == (N, D) and clusters.shape == (K, D)

    nc = _get_program()

    w1half = (-2.0 * clusters.T).astype(np.float32)
    w1 = np.ascontiguousarray(np.vstack([w1half, np.ones((D, K), np.float32)]))
    csq1 = (1.0 + np.sum(clusters * clusters, axis=1, dtype=np.float32)).reshape(1, K)
    csq = np.tile(csq1, (1, 2))
    consts = {
        "w1": w1,
        "csq": np.ascontiguousarray(csq.astype(np.float32)),
        "ones1": np.ones((1, 128), dtype=np.float32),
        "ident": np.eye(128, dtype=np.float32),
    }
    in_maps = [
        {"x": np.ascontiguousarray(x[i * NS : (i + 1) * NS]), **consts}
        for i in range(NCORES)
    ]
    res = run_bass_kernel_spmd(nc, in_maps, core_ids=list(range(NCORES)))
    out = np.concatenate([res.results[i]["q"] for i in range(NCORES)], axis=0)
    return out


# revision 11
# speedup vs baseline: 31810.4756x; 1.8733x over previous
"""Trainium2 Bass kernel for nn_ClusteringLayer (vq_codebook).

q[n,k] = t / sum_k t,  t = 1/(1 + ||x_n - c_k||^2)   (Student-t, alpha=1)

Strategy (8 NeuronCores, data-parallel over N):
  - shard x along N (32768 rows/core); replicate the (256,64) cluster table.
  - per core, process 1024-row chunks: 8 subtiles of 128 rows.
  - squared distances via PE matmuls accumulating into PSUM:
      S = (1+|c|^2)[k] + |x|^2[n] - 2 x.c
      mm1: lhsT=xT       rhs=-2*c^T      (K=64)
      mm2: lhsT=(x^2)T   rhs=ones[64,256](K=64)   -> adds |x|^2[n]
      mm3: lhsT=ones[1,128] rhs=(1+|c|^2)[1,256] (K=1)
    xT tiles come from PE pair-transposes ([128,128] -> PSUM), copied to
    SBUF by ScalarE (plain copy -> xT, Square-activation copy -> (x^2)T).
  - q_unnorm = 1/S via single-instruction DVE approx reciprocal (PSUM src).
  - rowsums on DVE (3D tensor_reduce), exact DVE reciprocal of the sums,
    per-row scale via tensor_scalar (2x mode), DMA out.
"""

import sys

sys.path.insert(0, "/opt/trn_rl_repo")

import numpy as np

N, D, K = 262144, 64, 256
NCORES = 8
NS = N // NCORES  # rows per core
CHUNK = 1024      # rows per chunk
G = 8             # subtiles (of 128 rows) per chunk
NCHUNK = NS // CHUNK

_CACHE = {}


def _build_program(loop_reps=None, mode="full"):
    import concourse.bacc as bacc
    import concourse.tile as tile
    from concourse import mybir

    f32 = mybir.dt.float32
    nc = bacc.Bacc("TRN2", target_bir_lowering=False, debug=False)

    f32r = mybir.dt.float32r
    x_ap = nc.dram_tensor("x", [NS, D], f32r, kind="ExternalInput").ap()
    w1_ap = nc.dram_tensor("w1", [128, K], f32r, kind="ExternalInput").ap()
    bf16 = mybir.dt.bfloat16
    csq_ap = nc.dram_tensor("csq", [3, 2 * K], bf16, kind="ExternalInput").ap()
    ones1_ap = nc.dram_tensor("ones1", [3, 128], bf16, kind="ExternalInput").ap()
    ident_ap = nc.dram_tensor("ident", [128, 128], f32r, kind="ExternalInput").ap()
    q_ap = nc.dram_tensor("q", [NS, K], f32, kind="ExternalOutput").ap()

    with tile.TileContext(nc) as tc:
        if loop_reps is None:
            _body(nc, tc, mybir, x_ap, w1_ap, csq_ap, ones1_ap, ident_ap, q_ap, mode=mode)
        else:
            with tc.For_i(0, loop_reps, 1):
                _body(nc, tc, mybir, x_ap, w1_ap, csq_ap, ones1_ap, ident_ap, q_ap, mode=mode)
    nc.compile()
    return nc


def _body(nc, tc, mybir, x_ap, w1_ap, csq_ap, ones1_ap, ident_ap, q_ap, mode="full"):
    from contextlib import ExitStack

    f32 = mybir.dt.float32
    ctx = ExitStack()
    with ctx:
        f32r = mybir.dt.float32r
        consts = ctx.enter_context(tc.tile_pool(name="consts", bufs=1))
        w1 = consts.tile([128, K], f32r)   # [-2 c^T ; ones(64,256)]
        nc.sync.dma_start(w1[:], w1_ap[:])
        bf16 = mybir.dt.bfloat16
        csq2 = consts.tile([3, 2 * K], bf16)
        nc.sync.dma_start(csq2[:], csq_ap[:])
        ones1 = consts.tile([3, 128], bf16)
        nc.sync.dma_start(ones1[:], ones1_ap[:])
        ident = consts.tile([128, 128], f32r)
        nc.sync.dma_start(ident[:], ident_ap[:])

        xp = ctx.enter_context(tc.tile_pool(name="xp", bufs=3))
        tpp = ctx.enter_context(tc.tile_pool(name="tpp", bufs=3, space="PSUM"))
        lp = ctx.enter_context(tc.tile_pool(name="lp", bufs=3))
        qp = ctx.enter_context(tc.tile_pool(name="qp", bufs=2, space="PSUM"))
        qup = ctx.enter_context(tc.tile_pool(name="qup", bufs=3))
        rsp = ctx.enter_context(tc.tile_pool(name="rsp", bufs=2))
        qop = ctx.enter_context(tc.tile_pool(name="qop", bufs=2))

        for c in range(NCHUNK):
            r0 = c * CHUNK
            # XB holds per subtile j a [x_j | x_j^2] 128-column block.
            xb = xp.tile([128, G * 128], f32r)  # [128, 1024]
            xb3 = xb[:].rearrange("p (g td) -> p g td", td=128)
            nc.sync.dma_start(
                xb3[:, :, 0:64],
                x_ap[r0 : r0 + CHUNK, :].rearrange("(p g) d -> p g d", p=128),
            )
            if mode in ("full", "pe"):
                nc.scalar.activation(
                    xb3[:, :, 64:128], xb3[:, :, 0:64],
                    mybir.ActivationFunctionType.Square,
                )
            rs = rsp.tile([128, G], f32)
            rr = rsp.tile([128, G], f32)
            qout = qop.tile([128, G * K], f32)  # [128, 2048]
            qus = []
            for hc in range(2):  # half-chunks of 4 subtiles
                psq = qp.tile([128, 4 * K], f32)  # [128, 1024] = 2 banks
                if mode == "dma":
                    continue
                # start=True clears has_written for the WHOLE bank: open each
                # bank group with one full-bank (1+|c|^2) rank-1 matmul.
                for b in range(2):
                    nc.tensor.matmul(
                        psq[:, 512 * b : 512 * (b + 1)], ones1[:], csq2[:],
                        start=True, stop=(mode == "dve"), skip_group_check=True,
                    )
                if mode in ("full", "pe"):
                    for jl in range(4):
                        j = 4 * hc + jl
                        pst = tpp.tile([128, 128], f32r)
                        nc.tensor.transpose(
                            pst[:], xb[:, 128 * j : 128 * (j + 1)], ident[:]
                        )
                        lt = lp.tile([128, 128], f32r)
                        nc.scalar.copy(lt[:], pst[:])
                        nc.tensor.matmul(
                            psq[:, K * jl : K * (jl + 1)], lt[:], w1[:],
                            start=False, stop=(jl >= 2), skip_group_check=True,
                        )
                if mode in ("full", "dve"):
                    qu = qup.tile([128, 4 * K], f32)
                    nc.vector.reciprocal_approx_fast(out=qu[:], in_=psq[:])
                    nc.vector.tensor_reduce(
                        rs[:, 4 * hc : 4 * hc + 4],
                        qu[:].rearrange("p (s k) -> p s k", k=K),
                        axis=mybir.AxisListType.X,
                        op=mybir.AluOpType.add,
                    )
                    qus.append(qu)
            if mode in ("full", "dve"):
                nc.vector.reciprocal(rr[:], rs[:])
                for j in range(G):
                    hc, jl = j // 4, j % 4
                    nc.vector.tensor_scalar_mul(
                        qout[:, K * j : K * (j + 1)],
                        qus[hc][:, K * jl : K * (jl + 1)],
                        rr[:, j : j + 1],
                    )
            else:
                nc.vector.memset(qout[:], 0.25)
            nc.sync.dma_start(
                q_ap[r0 : r0 + CHUNK, :].rearrange("(p g) k -> p (g k)", p=128),
                qout[:],
            )


def _get_program():
    if "nc" not in _CACHE:
        _CACHE["nc"] = _build_program()
    return _CACHE["nc"]


def kernel(x, clusters):
    from concourse.bass_utils import run_bass_kernel_spmd

    x = np.ascontiguousarray(np.asarray(x, dtype=np.float32))
    clusters = np.ascontiguousarray(np.asarray(clusters, dtype=np.float32))
    assert x.shape == (N, D) and clusters.shape == (K, D)

    nc = _get_program()

    w1half = (-2.0 * clusters.T).astype(np.float32)
    w1 = np.ascontiguousarray(np.vstack([w1half, np.ones((D, K), np.float32)]))
    csq1 = (1.0 + np.sum(clusters * clusters, axis=1, dtype=np.float32)).reshape(1, K)
    csq = np.tile(csq1, (1, 2))
    import ml_dtypes
    c_hi = csq.astype(ml_dtypes.bfloat16)
    r1 = csq - c_hi.astype(np.float32)
    c_mid = r1.astype(ml_dtypes.bfloat16)
    r2 = r1 - c_mid.astype(np.float32)
    c_lo = r2.astype(ml_dtypes.bfloat16)
    csq3 = np.ascontiguousarray(np.vstack([c_hi, c_mid, c_lo]))
    consts = {
        "w1": w1,
        "csq": csq3,
        "ones1": np.ones((3, 128), dtype=ml_dtypes.bfloat16),
        "ident": np.eye(128, dtype=np.float32),
    }
    in_maps = [
        {"x": np.ascontiguousarray(x[i * NS : (i + 1) * NS]), **consts}
        for i in range(NCORES)
    ]
    res = run_bass_kernel_spmd(nc, in_maps, core_ids=list(range(NCORES)))
    out = np.concatenate([res.results[i]["q"] for i in range(NCORES)], axis=0)
    return out
